# revision 3
# baseline (speedup 1.0000x reference)
"""GATv2 4-layer + MLP head on 8 Trainium2 NeuronCores (Bass/Tile) — v2.

Strategy: partition destination nodes across the 8 cores (1250 dst nodes
each, stored in 10 blocks of 128-row slots with 125 used). Each layer:
  node phase : xl/xr tables for the core's 1250 nodes via bf16 matmuls
               (weights resident in SBUF all run); an extra matmul column
               computes a = 0.6*sum_h s_h xl'_h per node (s = sign(att),
               |att| folded into the tables), stored at col 512 of the
               640-col xl rows. AllGather the xl table.
  edge phase : edges bucketed by dst into 10 blocks; per 128-edge tile:
               dma_gather xl[src] (640 cols) and xr[dst] (512 cols);
               u = xl+xr (DVE); e = 0.6L + 0.4(P-N) with
               P = sum_pos |u| (ScalarE Abs+accum over first kp cols),
               N = sum_neg |u| (DVE abs-reduce with negate), and the
               per-dst linear part of L cancels in segment softmax so
               only a_src is needed (rides the gather);
               p = exp(0.4*t + a) fused in one ScalarE Exp (bias AP);
               S = (iota == dst)*p built on DVE (one tensor_scalar);
               PE matmuls accumulate sum(p*xl[src]) and sum(p) per dst;
               h = relu(num/s); h stays in SBUF and is transposed
               per block (SBUF->SBUF DMA transpose) into the next
               layer's feature-major tiles.
MLP head feature-major; softmax via sigmoid of the logit difference.
The |att| scaling is undone by folding 1/|att| into the next layer's
weight rows on device; column sign permutations (pos-att first) are
index-only host work.
"""
import sys

sys.path.insert(0, "/opt/trn_rl_repo")

from contextlib import ExitStack

import numpy as np
import ml_dtypes

import concourse.bass as bass
import concourse.bacc as bacc
import concourse.tile as tile
from concourse import mybir
from concourse.bass_utils import run_bass_kernel_spmd

bf16 = mybir.dt.bfloat16
f32 = mybir.dt.float32
i16 = mybir.dt.int16
AF = mybir.ActivationFunctionType
ALU = mybir.AluOpType
AX = mybir.AxisListType
ts = bass.ts
npbf = ml_dtypes.bfloat16

N, E, DIN, H = 10000, 80000, 1024, 512
NEG = 0.2
NC = 8
NBLK = 10              # dst blocks per core
BLK = 125              # dst nodes per block
SLOT = 128             # rows per block slot (125 used)
NPAD = NBLK * SLOT     # 1280 rows per core
XLW = 640              # xl row width: 512 feats + a col + pad (1280B)
PADV = 200.0           # dst sentinel for pad edges (matches no iota col)


# ---------------------------------------------------------------- host prep
def _prep_edges(edge_index):
    src = np.concatenate([edge_index[0], np.arange(N)]).astype(np.int64)
    dst = np.concatenate([edge_index[1], np.arange(N)]).astype(np.int64)
    deg = np.bincount(dst, minlength=N)
    NBUCK = NC * NBLK
    order = np.argsort(-deg, kind="stable")
    bucket_edges = np.zeros(NBUCK, np.int64)
    bucket_nodes = [[] for _ in range(NBUCK)]
    import heapq
    heap = [(0, kk) for kk in range(NBUCK)]
    heapq.heapify(heap)
    for g in order:
        while True:
            w, kk = heapq.heappop(heap)
            if len(bucket_nodes[kk]) < BLK:
                break
        bucket_nodes[kk].append(int(g))
        bucket_edges[kk] = w + int(deg[g])
        if len(bucket_nodes[kk]) < BLK:
            heapq.heappush(heap, (int(bucket_edges[kk]), kk))
    # node -> (core, block, j); global gather row = c*NPAD + b*SLOT + j
    assign = [[] for _ in range(NC)]   # assign[c][b*BLK+i] = node
    pos = np.empty(N, np.int64)
    for c in range(NC):
        for b in range(NBLK):
            nodes = bucket_nodes[c * NBLK + b]
            assign[c].extend(nodes)
            for j, g in enumerate(nodes):
                pos[g] = c * NPAD + b * SLOT + j
    assign = [np.array(a, np.int64) for a in assign]
    percore = []
    for c in range(NC):
        sel = (pos[dst] // NPAD) == c
        s_ = pos[src[sel]]
        dloc = pos[dst[sel]] - c * NPAD          # b*SLOT + j
        blocks = []
        for b in range(NBLK):
            m = (dloc // SLOT) == b
            blocks.append((s_[m], dloc[m] - b * SLOT))   # (global row, j)
        percore.append(blocks)
    TBs = tuple(max(max(-(-len(percore[c][b][0]) // 128), 1) for c in range(NC))
                for b in range(NBLK))
    cum = np.concatenate([[0], np.cumsum(TBs)]).astype(int)
    NT = int(cum[-1])
    EPAD = NT * 128
    cores = []
    for c in range(NC):
        src16 = np.zeros(EPAD, np.int16)
        dst16 = np.zeros(EPAD, np.int16)
        dstval = np.full((128, NT), PADV, np.float32)
        for b in range(NBLK):
            s, j = percore[c][b]
            n = len(s)
            base = int(cum[b]) * 128
            src16[base:base + n] = s
            dst16[base:base + n] = b * SLOT + j
            dst16[base + n:(int(cum[b]) + TBs[b]) * 128] = b * SLOT  # valid row; dstval sentinel zeroes it
            for i in range(n):
                dstval[i % 128, int(cum[b]) + i // 128] = j[i]
        def wrap(a):
            w = a.reshape(-1, 16).T.copy()
            return np.tile(w, (8, 1)).copy()
        cores.append(dict(src16=wrap(src16), dst16=wrap(dst16),
                          dstval=np.ascontiguousarray(dstval)))
    return TBs, cores, assign


# -------------------------------------------------------------- bass program
def _build(TBs, KP, use_bias, single_core=False):
    TBs = tuple(TBs)
    TBMAX = max(TBs)
    cum = [0]
    for t in TBs:
        cum.append(cum[-1] + t)
    NT = cum[-1]
    nc = bacc.Bacc("TRN2", num_swdge_queues=4)
    P = nc.declare_dram_parameter
    x_in = P("x", [NPAD, DIN], f32, isOutput=False)
    wl_in, wr_in, att_in, bl_in, br_in, bb_in = [], [], [], [], [], []
    for l in range(4):
        din = DIN if l == 0 else H
        wl_in.append(P(f"wl{l}", [din, H], f32, isOutput=False))
        wr_in.append(P(f"wr{l}", [din, H], f32, isOutput=False))
        att_in.append(P(f"att{l}", [1, H], f32, isOutput=False))
        bl_in.append(P(f"bl{l}", [1, H], f32, isOutput=False))
        br_in.append(P(f"br{l}", [1, H], f32, isOutput=False))
        bb_in.append(P(f"bb{l}", [1, H], f32, isOutput=False))
    lw1_in = P("lw1", [H, H], f32, isOutput=False)
    lb1_in = P("lb1", [1, H], f32, isOutput=False)
    lw2_in = P("lw2", [H, 256], f32, isOutput=False)
    lb2_in = P("lb2", [1, 256], f32, isOutput=False)
    lw3_in = P("lw3", [256, 2], f32, isOutput=False)
    lb3_in = P("lb3", [2, 1], f32, isOutput=False)
    srcidx_in = P("srcidx", [128, NT * 8], i16, isOutput=False)
    dstidx_in = P("dstidx", [128, NT * 8], i16, isOutput=False)
    dstval_in = P("dstval", [128, NT], f32, isOutput=False)
    iota_in = P("iota", [128, BLK], f32, isOutput=False)
    sgn_in = P("sgn", [2, 1], f32, isOutput=False)
    logitsT_out = P("logitsT", [2, NPAD], f32, isOutput=True)
    probs0_out = P("probs0", [1, NPAD], f32, isOutput=True)
    probs1_out = P("probs1", [1, NPAD], f32, isOutput=True)

    x_bf = nc.dram_tensor("x_bf", [NPAD, DIN], bf16)
    xl_loc, xl_full, xr_dr = [], [], []
    for l in range(4):
        xl_loc.append(nc.dram_tensor(f"xlloc{l}", [NPAD, XLW], bf16))
        xl_full.append(nc.dram_tensor(f"xlfull{l}", [NC * NPAD, XLW], bf16,
                                      addr_space="Shared"))
        xr_dr.append(nc.dram_tensor(f"xr{l}", [NPAD, H], bf16))
    h_dr = [nc.dram_tensor(f"h{l}", [NPAD, H], bf16) for l in range(4)]

    with tile.TileContext(nc) as tc, ExitStack() as ctx:
        wp = ctx.enter_context(tc.tile_pool(name="wp", bufs=1))
        np_ = ctx.enter_context(tc.tile_pool(name="np", bufs=2))
        ep = ctx.enter_context(tc.tile_pool(name="ep", bufs=3))
        gp = ctx.enter_context(tc.tile_pool(name="gp", bufs=2))
        ps = ctx.enter_context(tc.tile_pool(name="ps", bufs=2, space="PSUM"))

        # ---------------- constants, indices ----------------
        ones128 = wp.tile([128, 1], bf16, tag="ones128")
        nc.vector.memset(ones128[:, :], 1.0)
        onesrow = wp.tile([1, 128], bf16, tag="onesrow")
        nc.vector.memset(onesrow[:1, :], 1.0)
        sgn = wp.tile([2, 1], f32, tag="sgn")
        nc.sync.dma_start(out=sgn[:2, :], in_=sgn_in[:, :])
        srcidx = wp.tile([128, NT * 8], i16, tag="srcidx")
        nc.sync.dma_start(out=srcidx[:, :], in_=srcidx_in[:, :])
        dstidx = wp.tile([128, NT * 8], i16, tag="dstidx")
        nc.sync.dma_start(out=dstidx[:, :], in_=dstidx_in[:, :])
        dstval = wp.tile([128, NT], f32, tag="dstval")
        nc.sync.dma_start(out=dstval[:, :], in_=dstval_in[:, :])
        iotaf = wp.tile([128, BLK], f32, tag="iotaf")
        nc.sync.dma_start(out=iotaf[:, :], in_=iota_in[:, :])
        iota = wp.tile([128, BLK], bf16, tag="iota")
        nc.vector.tensor_copy(iota[:, :], iotaf[:, :])

        # ---------------- weight prep (SBUF-resident) ----------------
        attb, blb_row, brb_row, recipcol = [], [], [], []
        for l in range(4):
            ab = wp.tile([128, H], f32, tag=f"attb{l}")
            nc.sync.dma_start(out=ab[:, :], in_=att_in[l][:, :].broadcast_to((128, H)))
            nc.scalar.activation(ab[:, :], ab[:, :], AF.Abs)
            nc.vector.tensor_scalar_max(ab[:, :], ab[:, :], 1e-30)
            attb.append(ab)
            rc = wp.tile([128, H // 128], f32, tag=f"rc{l}")
            nc.sync.dma_start(out=rc[:, :],
                              in_=att_in[l][0, :].rearrange("(k p) -> p k", p=128))
            nc.scalar.activation(rc[:, :], rc[:, :], AF.Abs)
            nc.vector.tensor_scalar_max(rc[:, :], rc[:, :], 1e-30)
            rcr = wp.tile([128, H // 128], f32, tag=f"rcr{l}")
            nc.vector.reciprocal(rcr[:, :], rc[:, :])
            recipcol.append(rcr)
            if use_bias:
                trow = np_.tile([1, H], f32, tag="brow_ld", bufs=1)
                nc.sync.dma_start(out=trow[:1, :], in_=bl_in[l][:, :])
                trow2 = np_.tile([1, H], f32, tag="brow_ld2", bufs=1)
                nc.sync.dma_start(out=trow2[:1, :], in_=bb_in[l][:, :])
                tsum = np_.tile([1, H], f32, tag="brow_sum", bufs=1)
                nc.vector.tensor_add(tsum[:1, :], trow[:1, :], trow2[:1, :])
                blr = wp.tile([1, H], bf16, tag=f"blb{l}")
                nc.vector.tensor_mul(blr[:1, :], tsum[:1, :], ab[0:1, :])
                blb_row.append(blr)
                trow3 = np_.tile([1, H], f32, tag="brow_ld3", bufs=1)
                nc.sync.dma_start(out=trow3[:1, :], in_=br_in[l][:, :])
                tdif = np_.tile([1, H], f32, tag="brow_dif", bufs=1)
                nc.vector.tensor_sub(tdif[:1, :], trow3[:1, :], trow2[:1, :])
                brr = wp.tile([1, H], bf16, tag=f"brb{l}")
                nc.vector.tensor_mul(brr[:1, :], tdif[:1, :], ab[0:1, :])
                brb_row.append(brr)
            else:
                blb_row.append(None)
                brb_row.append(None)

        # GAT weights -> SBUF, col-scaled by |att_l|, rows by 1/|att_{l-1}|
        wld_sb, wrd_sb = [], []
        for l in range(4):
            din = DIN if l == 0 else H
            nk0 = din // 128
            for W_in, lst, nm in ((wl_in[l], wld_sb, "wl"), (wr_in[l], wrd_sb, "wr")):
                wsb = wp.tile([128, nk0, H], bf16, tag=f"{nm}d{l}")
                for k0 in range(0, nk0, 2):
                    kw = min(2, nk0 - k0)
                    wt = np_.tile([128, 2, H], f32, tag="wprep", bufs=1)
                    nc.sync.dma_start(
                        out=wt[:, :kw, :],
                        in_=W_in[k0 * 128:(k0 + kw) * 128, :].rearrange(
                            "(k p) h -> p k h", p=128))
                    for kk in range(kw):
                        k = k0 + kk
                        if l == 0:
                            nc.vector.tensor_mul(wsb[:, k, :], wt[:, kk, :], attb[l][:, :])
                        else:
                            wt2 = np_.tile([128, H], f32, tag="wprep2", bufs=2)
                            nc.vector.tensor_mul(wt2[:, :], wt[:, kk, :], attb[l][:, :])
                            nc.vector.tensor_scalar_mul(wsb[:, k, :], wt2[:, :],
                                                        recipcol[l - 1][:, k:k + 1])
                lst.append(wsb)

        # MLP weights
        lw1_dev = []
        for k in range(4):
            wt = np_.tile([128, H], f32, tag="wprep1", bufs=1)
            nc.sync.dma_start(out=wt[:, :], in_=lw1_in[ts(k, 128), :])
            wdev = wp.tile([128, H], bf16, tag=f"lw1_{k}")
            nc.vector.tensor_scalar_mul(wdev[:, :], wt[:, :], recipcol[3][:, k:k + 1])
            lw1_dev.append(wdev)
        lw2_dev = []
        for k in range(4):
            wt = np_.tile([128, 256], f32, tag="wprep1", bufs=1)
            nc.sync.dma_start(out=wt[:, :], in_=lw2_in[ts(k, 128), :])
            wdev = wp.tile([128, 256], bf16, tag=f"lw2_{k}")
            nc.vector.tensor_copy(wdev[:, :], wt[:, :])
            lw2_dev.append(wdev)
        lw3_dev = []
        for k in range(2):
            wt3 = np_.tile([128, 2], f32, tag="wprep3", bufs=1)
            nc.sync.dma_start(out=wt3[:, :], in_=lw3_in[ts(k, 128), :])
            wdev = wp.tile([128, 2], bf16, tag=f"lw3_{k}")
            nc.vector.tensor_copy(wdev[:, :], wt3[:, :])
            lw3_dev.append(wdev)
        lb1col = wp.tile([128, 4], f32, tag="lb1c")
        nc.sync.dma_start(out=lb1col[:, :], in_=lb1_in[0, :].rearrange("(k p) -> p k", p=128))
        lb2col = wp.tile([128, 2], f32, tag="lb2c")
        nc.sync.dma_start(out=lb2col[:, :], in_=lb2_in[0, :].rearrange("(k p) -> p k", p=128))
        lb3col = wp.tile([2, 1], f32, tag="lb3c")
        nc.sync.dma_start(out=lb3col[:2, :], in_=lb3_in[:, :])

        # x: fp32 -> bf16 cast for transposes
        nc.gpsimd.dma_start(out=x_bf[:, :], in_=x_in[:, :])

        # hT tiles (feature-major activations), persistent tags
        nkmax = DIN // 128
        hT = [np_.tile([128, NPAD], bf16, tag=f"hT{k}", name=f"hT{k}", bufs=1)
              for k in range(nkmax)]
        for k in range(nkmax):
            nc.sync.dma_start(out=hT[k][:, :], in_=x_bf[:, ts(k, 128)], transpose=True)

        # ---------------- layers (node l+1 interleaved into edge l) ----
        def node_mm(lw, m):
            """PE part of the table build for chunk m of layer lw."""
            nkw = (DIN if lw == 0 else H) // 128
            sl = slice(m * 128, (m + 1) * 128)
            pxl = ps.tile([128, H], f32, tag="pnl")
            pxr = ps.tile([128, H], f32, tag="pnr")
            last = not use_bias
            for k in range(nkw):
                st = (k == 0)
                sp = last and (k == nkw - 1)
                nc.tensor.matmul(pxl[:, :], hT[k][:, sl], wld_sb[lw][:, k, :],
                                 start=st, stop=sp, skip_group_check=True)
                nc.tensor.matmul(pxr[:, :], hT[k][:, sl], wrd_sb[lw][:, k, :],
                                 start=st, stop=sp, skip_group_check=True)
            if use_bias:
                nc.tensor.matmul(pxl[:, :], onesrow[:1, :], blb_row[lw][:1, :],
                                 start=False, stop=True, skip_group_check=True)
                nc.tensor.matmul(pxr[:, :], onesrow[:1, :], brb_row[lw][:1, :],
                                 start=False, stop=True, skip_group_check=True)
            return pxl, pxr

        def node_stage(lw, m, pxl, pxr):
            """Staging (copies + a-col + DMA out) for chunk m of layer lw."""
            kpw = KP[lw]
            xl_sb = np_.tile([128, XLW], bf16, tag="xlsb", bufs=2)
            xr_sb = np_.tile([128, H], bf16, tag="xrsb", bufs=2)
            A1 = np_.tile([128, 1], f32, tag="A1", bufs=2)
            A2 = np_.tile([128, 1], f32, tag="A2", bufs=2)
            if kpw > 0:
                nc.scalar.activation(xl_sb[:, :kpw], pxl[:, :kpw], AF.Copy,
                                     accum_out=A1[:, :])
            else:
                nc.vector.memset(A1[:, :], 0.0)
            if kpw < H:
                nc.scalar.activation(xl_sb[:, kpw:H], pxl[:, kpw:], AF.Copy,
                                     accum_out=A2[:, :])
            else:
                nc.vector.memset(A2[:, :], 0.0)
            dA = np_.tile([128, 1], f32, tag="dA", bufs=2)
            nc.vector.tensor_sub(dA[:, :], A1[:, :], A2[:, :])
            nc.vector.tensor_scalar_mul(xl_sb[:, H:H + 1], dA[:, :], 0.6)
            nc.vector.tensor_copy(xr_sb[:, :], pxr[:, :])
            nc.sync.dma_start(out=xl_loc[lw][m * 128:(m + 1) * 128, :],
                              in_=xl_sb[:, :])
            nc.sync.dma_start(out=xr_dr[lw][m * 128:(m + 1) * 128, :],
                              in_=xr_sb[:, :])

        def node_chunk(lw, m):
            pxl, pxr = node_mm(lw, m)
            node_stage(lw, m, pxl, pxr)

        def all_gather(lw):
            if single_core:
                nc.sync.dma_start(out=xl_full[lw][0:NPAD, :], in_=xl_loc[lw][:, :])
            else:
                nc.gpsimd.collective_compute(
                    "AllGather", ALU.bypass,
                    replica_groups=[list(range(NC))],
                    ins=[xl_loc[lw][:, :]], outs=[xl_full[lw][:, :]],
                )

        for m in range(NBLK):
            node_chunk(0, m)
        all_gather(0)

        for l in range(4):
            kp = KP[l]
            for b in range(NBLK):
                if l < 3 and b >= 5:
                    pxlr = node_mm(l + 1, b - 5)
                TB = TBs[b]
                c0 = cum[b]
                nidx = TB * 128
                TBh = (TB + 1) // 2
                xlg = gp.tile([128, TBMAX, XLW], bf16, tag="xlg")
                nc.gpsimd.dma_gather(
                    out_ap=xlg[:, :TBh, :], in_ap=xl_full[l][:, :],
                    idxs_ap=srcidx[:, c0 * 8:(c0 + TBh) * 8],
                    num_idxs=TBh * 128, num_idxs_reg=TBh * 128, elem_size=XLW,
                    single_packet=False)
                if TB > TBh:
                    nc.gpsimd.dma_gather(
                        out_ap=xlg[:, TBh:TB, :], in_ap=xl_full[l][:, :],
                        idxs_ap=srcidx[:, (c0 + TBh) * 8:(c0 + TB) * 8],
                        num_idxs=(TB - TBh) * 128, num_idxs_reg=(TB - TBh) * 128,
                        elem_size=XLW, single_packet=False, queue_num=2)
                xrg = gp.tile([128, TBMAX, H], bf16, tag="xrg")
                nc.gpsimd.dma_gather(
                    out_ap=xrg[:, :TBh, :], in_ap=xr_dr[l][:, :],
                    idxs_ap=dstidx[:, c0 * 8:(c0 + TBh) * 8],
                    num_idxs=TBh * 128, num_idxs_reg=TBh * 128, elem_size=H,
                    single_packet=False, queue_num=1)
                if TB > TBh:
                    nc.gpsimd.dma_gather(
                        out_ap=xrg[:, TBh:TB, :], in_ap=xr_dr[l][:, :],
                        idxs_ap=dstidx[:, (c0 + TBh) * 8:(c0 + TB) * 8],
                        num_idxs=(TB - TBh) * 128, num_idxs_reg=(TB - TBh) * 128,
                        elem_size=H, single_packet=False, queue_num=3)
                ubuf = gp.tile([128, TBMAX, H], bf16, tag="ubuf")
                Pcol = ep.tile([128, TBMAX], f32, tag="Pcol", bufs=2)
                nNc = ep.tile([128, TBMAX], f32, tag="nNc", bufs=2)
                tcol = ep.tile([128, TBMAX], f32, tag="tcol", bufs=2)
                tc2 = ep.tile([128, TBMAX], f32, tag="tc2", bufs=2)
                pbuf = ep.tile([128, TBMAX], f32, tag="pbuf", bufs=2)
                Sbuf = ep.tile([128, TBMAX, BLK], bf16, tag="Sbuf", bufs=1)
                pf = ps.tile([128, H], f32, tag="pf")
                ps1 = ps.tile([128, 1], f32, tag="pcol1")
                for t in range(TB):
                    nc.vector.tensor_add(ubuf[:, t, :], xlg[:, t, :H], xrg[:, t, :])
                for t in range(TB):
                    scratch = ep.tile([128, H], bf16, tag="scr", bufs=1)
                    if kp > 0:
                        nc.scalar.activation(scratch[:, :kp], ubuf[:, t, :kp], AF.Abs,
                                             accum_out=Pcol[:, t:t + 1])
                for t in range(TB):
                    if kp < H:
                        nc.vector.tensor_reduce(nNc[:, t:t + 1], ubuf[:, t, kp:],
                                                AX.X, ALU.add,
                                                apply_absolute_value=True, negate=True)
                if kp == 0:
                    nc.vector.memset(Pcol[:, :TB], 0.0)
                if kp == H:
                    nc.vector.memset(nNc[:, :TB], 0.0)
                nc.vector.tensor_add(tcol[:, :TB], Pcol[:, :TB], nNc[:, :TB])
                nc.vector.scalar_tensor_tensor(tc2[:, :TB], xlg[:, :TB, H:H + 1],
                                               2.5, tcol[:, :TB],
                                               ALU.mult, ALU.add)
                nc.scalar.activation(pbuf[:, :TB], tc2[:, :TB], AF.Exp, scale=0.4)
                for t in range(TB):
                    nc.vector.tensor_scalar(Sbuf[:, t, :], iota[:, :],
                                            dstval[:, c0 + t:c0 + t + 1],
                                            pbuf[:, t:t + 1],
                                            ALU.is_equal, ALU.mult)
                for t in range(TB):
                    nc.tensor.matmul(pf[:BLK, :], Sbuf[:, t, :], xlg[:, t, :H],
                                     start=(t == 0), stop=(t == TB - 1),
                                     skip_group_check=True)
                    nc.tensor.matmul(ps1[:BLK, :1], Sbuf[:, t, :], ones128[:, :1],
                                     start=(t == 0), stop=(t == TB - 1),
                                     skip_group_check=True)
                srec = ep.tile([128, 1], f32, tag="srec")
                nc.vector.reciprocal(srec[:BLK, :], ps1[:BLK, :1])
                hb = ep.tile([128, H], bf16, tag="hb", bufs=2)
                nc.scalar.activation(hb[:BLK, :], pf[:BLK, :], AF.Relu,
                                     scale=srec[:BLK, :])
                nc.sync.dma_start(out=h_dr[l][b * SLOT:b * SLOT + BLK, :],
                                  in_=hb[:BLK, :])
                if b == 4:
                    # first-half feature-major tiles; chunks 0-4 build during
                    # blocks 5-9
                    for k in range(4):
                        nc.sync.dma_start(out=hT[k][:, :640],
                                          in_=h_dr[l][:640, ts(k, 128)],
                                          transpose=True)
                if l < 3 and b >= 5:
                    node_stage(l + 1, b - 5, pxlr[0], pxlr[1])
            for k in range(4):
                nc.sync.dma_start(out=hT[k][:, 640:],
                                  in_=h_dr[l][640:, ts(k, 128)], transpose=True)
            if l < 3:
                for m in range(5, NBLK):
                    node_chunk(l + 1, m)
                all_gather(l + 1)

        # ---------------- MLP head (feature-major) ----------------
        jchunks = [(0, 512), (512, 512), (1024, 256)]
        h1T = [np_.tile([128, NPAD], bf16, tag=f"h1T{m}", name=f"h1T{m}", bufs=1)
               for m in range(4)]
        for m in range(4):
            for j0, w in jchunks:
                pm = ps.tile([128, H], f32, tag="pf")
                for k in range(4):
                    nc.tensor.matmul(pm[:, :w], lw1_dev[k][:, ts(m, 128)],
                                     hT[k][:, j0:j0 + w], start=(k == 0), stop=(k == 3),
                                     skip_group_check=True)
                nc.scalar.activation(h1T[m][:, j0:j0 + w], pm[:, :w], AF.Relu,
                                     bias=lb1col[:, m:m + 1])
        h2T = [np_.tile([128, NPAD], bf16, tag=f"h2T{m}", name=f"h2T{m}", bufs=1)
               for m in range(2)]
        for m in range(2):
            for j0, w in jchunks:
                pm = ps.tile([128, H], f32, tag="pf")
                for k in range(4):
                    nc.tensor.matmul(pm[:, :w], lw2_dev[k][:, ts(m, 128)],
                                     h1T[k][:, j0:j0 + w], start=(k == 0), stop=(k == 3),
                                     skip_group_check=True)
                nc.scalar.activation(h2T[m][:, j0:j0 + w], pm[:, :w], AF.Relu,
                                     bias=lb2col[:, m:m + 1])
        logT = np_.tile([2, NPAD], f32, tag="logT", bufs=1)
        p0 = np_.tile([1, NPAD], f32, tag="p0", bufs=1)
        p1 = np_.tile([1, NPAD], f32, tag="p1", bufs=1)
        for j0, w in jchunks:
            pm = ps.tile([128, H], f32, tag="pf")
            for k in range(2):
                nc.tensor.matmul(pm[:2, :w], lw3_dev[k][:, :],
                                 h2T[k][:, j0:j0 + w], start=(k == 0), stop=(k == 1),
                                 skip_group_check=True)
            nc.scalar.activation(logT[:2, j0:j0 + w], pm[:2, :w], AF.Identity,
                                 bias=lb3col[:2, :])
            pd = ps.tile([128, H], f32, tag="pf")
            nc.tensor.matmul(pd[:1, :w], sgn[:2, :], logT[:2, j0:j0 + w],
                             start=True, stop=True, skip_group_check=True)
            nc.scalar.activation(p1[:1, j0:j0 + w], pd[:1, :w], AF.Sigmoid)
            nc.scalar.activation(p0[:1, j0:j0 + w], pd[:1, :w], AF.Sigmoid, scale=-1.0)
        nc.sync.dma_start(out=logitsT_out[:, :], in_=logT[:2, :])
        nc.sync.dma_start(out=probs0_out[:, :], in_=p0[:1, :])
        nc.sync.dma_start(out=probs1_out[:, :], in_=p1[:1, :])

    nc.compile()
    return nc


_CACHE = {}
_LAST_IN_MAPS = None


def _get_program(TBs, KP, use_bias):
    key = (tuple(TBs), tuple(KP), use_bias)
    if key not in _CACHE:
        _CACHE[key] = _build(TBs, KP, use_bias)
    return _CACHE[key]


def _run(inputs, trace=False):
    inp = {k: np.asarray(v) for k, v in inputs.items()}
    x = inp["x"].astype(np.float32)
    edge_index = inp["edge_index"].astype(np.int64)
    TBs, cores, assign = _prep_edges(edge_index)

    perms, KP = [], []
    for l in range(1, 5):
        att = inp[f"att{l}"].astype(np.float32)
        perm = np.argsort(att <= 0, kind="stable")
        perms.append(perm)
        KP.append(int((att > 0).sum()))
    use_bias = any(
        np.abs(inp[k]).max() > 0
        for k in ("bl1", "br1", "b1", "bl2", "br2", "b2",
                  "bl3", "br3", "b3", "bl4", "br4", "b4"))
    w_host = {}
    for l in range(4):
        rowp = perms[l - 1] if l > 0 else None
        for nm in ("wl", "wr"):
            W = inp[f"{nm}{l + 1}"].astype(np.float32)
            if rowp is not None:
                W = W[rowp, :]
            w_host[f"{nm}{l}"] = np.ascontiguousarray(W[:, perms[l]])
        for nm, key in (("att", "att"), ("bl", "bl"), ("br", "br"), ("bb", "b")):
            v = inp[f"{key}{l + 1}"].astype(np.float32)[perms[l]]
            w_host[f"{nm}{l}"] = v.reshape(1, H).copy()
    lw1 = np.ascontiguousarray(inp["lw1"].astype(np.float32)[perms[3], :])

    x_pad = np.zeros((NC, NPAD, DIN), np.float32)
    for c in range(NC):
        for b in range(NBLK):
            x_pad[c, b * SLOT:b * SLOT + BLK] = x[assign[c][b * BLK:(b + 1) * BLK]]

    iota = np.tile(np.arange(BLK, dtype=np.float32), (128, 1)).copy()
    ncprog = _get_program(TBs, KP, use_bias)
    in_maps = []
    for c in range(NC):
        m = {"x": x_pad[c],
             "lw1": lw1, "lb1": inp["lb1"].astype(np.float32).reshape(1, H),
             "lw2": inp["lw2"].astype(np.float32),
             "lb2": inp["lb2"].astype(np.float32).reshape(1, 256),
             "lw3": inp["lw3"].astype(np.float32),
             "lb3": inp["lb3"].astype(np.float32).reshape(2, 1),
             "sgn": np.array([[-1.0], [1.0]], np.float32),
             "srcidx": cores[c]["src16"], "dstidx": cores[c]["dst16"],
             "dstval": cores[c]["dstval"], "iota": iota}
        m.update(w_host)
        in_maps.append(m)

    global _LAST_IN_MAPS
    _LAST_IN_MAPS = in_maps
    res = run_bass_kernel_spmd(ncprog, in_maps, list(range(NC)), trace=trace)
    logits = np.empty((N, 2), np.float32)
    probs = np.empty((N, 2), np.float32)
    slot_rows = np.concatenate([np.arange(b * SLOT, b * SLOT + BLK)
                                for b in range(NBLK)])
    for c in range(NC):
        r = res.results[c]
        logits[assign[c]] = r["logitsT"].T[slot_rows]
        probs[assign[c], 0] = r["probs0"][0][slot_rows]
        probs[assign[c], 1] = r["probs1"][0][slot_rows]
    return (logits, probs), res


def kernel(**inputs):
    out, _ = _run(inputs, trace=False)
    return out


# revision 4
# speedup vs baseline: 5.8396x; 5.8396x over previous
"""GATv2 4-layer + MLP head on 8 Trainium2 NeuronCores (Bass/Tile) — v2.

Strategy: partition destination nodes across the 8 cores (1250 dst nodes
each, stored in 10 blocks of 128-row slots with 125 used). Each layer:
  node phase : xl/xr tables for the core's 1250 nodes via bf16 matmuls
               (weights resident in SBUF all run); an extra matmul column
               computes a = 0.6*sum_h s_h xl'_h per node (s = sign(att),
               |att| folded into the tables), stored at col 512 of the
               640-col xl rows. AllGather the xl table.
  edge phase : edges bucketed by dst into 10 blocks; per 128-edge tile:
               dma_gather xl[src] (640 cols) and xr[dst] (512 cols);
               u = xl+xr (DVE); e = 0.6L + 0.4(P-N) with
               P = sum_pos |u| (ScalarE Abs+accum over first kp cols),
               N = sum_neg |u| (DVE abs-reduce with negate), and the
               per-dst linear part of L cancels in segment softmax so
               only a_src is needed (rides the gather);
               p = exp(0.4*t + a) fused in one ScalarE Exp (bias AP);
               S = (iota == dst)*p built on DVE (one tensor_scalar);
               PE matmuls accumulate sum(p*xl[src]) and sum(p) per dst;
               h = relu(num/s); h stays in SBUF and is transposed
               per block (SBUF->SBUF DMA transpose) into the next
               layer's feature-major tiles.
MLP head feature-major; softmax via sigmoid of the logit difference.
The |att| scaling is undone by folding 1/|att| into the next layer's
weight rows on device; column sign permutations (pos-att first) are
index-only host work.
"""
import sys

sys.path.insert(0, "/opt/trn_rl_repo")

from contextlib import ExitStack

import numpy as np
import ml_dtypes

import concourse.bass as bass
import concourse.bacc as bacc
import concourse.tile as tile
from concourse import mybir
from concourse.bass_utils import run_bass_kernel_spmd

bf16 = mybir.dt.bfloat16
f32 = mybir.dt.float32
i16 = mybir.dt.int16
AF = mybir.ActivationFunctionType
ALU = mybir.AluOpType
AX = mybir.AxisListType
ts = bass.ts
npbf = ml_dtypes.bfloat16

N, E, DIN, H = 10000, 80000, 1024, 512
NEG = 0.2
NC = 8
NBLK = 10              # dst blocks per core
BLK = 125              # dst nodes per block
SLOT = 128             # rows per block slot (125 used)
NPAD = NBLK * SLOT     # 1280 rows per core
XLW = 640              # xl row width: 512 feats + a col + pad (1280B)
PADV = 200.0           # dst sentinel for pad edges (matches no iota col)


# ---------------------------------------------------------------- host prep
def _prep_edges(edge_index):
    src = np.concatenate([edge_index[0], np.arange(N)]).astype(np.int64)
    dst = np.concatenate([edge_index[1], np.arange(N)]).astype(np.int64)
    deg = np.bincount(dst, minlength=N)
    NBUCK = NC * NBLK
    order = np.argsort(-deg, kind="stable")
    bucket_edges = np.zeros(NBUCK, np.int64)
    bucket_nodes = [[] for _ in range(NBUCK)]
    import heapq
    heap = [(0, kk) for kk in range(NBUCK)]
    heapq.heapify(heap)
    for g in order:
        while True:
            w, kk = heapq.heappop(heap)
            if len(bucket_nodes[kk]) < BLK:
                break
        bucket_nodes[kk].append(int(g))
        bucket_edges[kk] = w + int(deg[g])
        if len(bucket_nodes[kk]) < BLK:
            heapq.heappush(heap, (int(bucket_edges[kk]), kk))
    # node -> (core, block, j); global gather row = c*NPAD + b*SLOT + j
    assign = [[] for _ in range(NC)]   # assign[c][b*BLK+i] = node
    pos = np.empty(N, np.int64)
    for c in range(NC):
        for b in range(NBLK):
            nodes = bucket_nodes[c * NBLK + b]
            assign[c].extend(nodes)
            for j, g in enumerate(nodes):
                pos[g] = c * NPAD + b * SLOT + j
    assign = [np.array(a, np.int64) for a in assign]
    percore = []
    for c in range(NC):
        sel = (pos[dst] // NPAD) == c
        s_ = pos[src[sel]]
        dloc = pos[dst[sel]] - c * NPAD          # b*SLOT + j
        blocks = []
        for b in range(NBLK):
            m = (dloc // SLOT) == b
            blocks.append((s_[m], dloc[m] - b * SLOT))   # (global row, j)
        percore.append(blocks)
    TBs = tuple(max(max(-(-len(percore[c][b][0]) // 128), 1) for c in range(NC))
                for b in range(NBLK))
    cum = np.concatenate([[0], np.cumsum(TBs)]).astype(int)
    NT = int(cum[-1])
    EPAD = NT * 128
    cores = []
    for c in range(NC):
        src16 = np.zeros(EPAD, np.int16)
        dst16 = np.zeros(EPAD, np.int16)
        dstval = np.full((128, NT), PADV, np.float32)
        for b in range(NBLK):
            s, j = percore[c][b]
            n = len(s)
            base = int(cum[b]) * 128
            src16[base:base + n] = s
            dst16[base:base + n] = b * SLOT + j
            dst16[base + n:(int(cum[b]) + TBs[b]) * 128] = b * SLOT  # valid row; dstval sentinel zeroes it
            for i in range(n):
                dstval[i % 128, int(cum[b]) + i // 128] = j[i]
        def wrap(a):
            w = a.reshape(-1, 16).T.copy()
            return np.tile(w, (8, 1)).copy()
        cores.append(dict(src16=wrap(src16), dst16=wrap(dst16),
                          dstval=np.ascontiguousarray(dstval)))
    return TBs, cores, assign


# -------------------------------------------------------------- bass program
def _build(TBs, KP, use_bias, single_core=False):
    TBs = tuple(TBs)
    TBMAX = max(TBs)
    cum = [0]
    for t in TBs:
        cum.append(cum[-1] + t)
    NT = cum[-1]
    nc = bacc.Bacc("TRN2", num_swdge_queues=2)
    P = nc.declare_dram_parameter
    x_in = P("x", [NPAD, DIN], f32, isOutput=False)
    wl_in, wr_in, att_in, bl_in, br_in, bb_in = [], [], [], [], [], []
    for l in range(4):
        din = DIN if l == 0 else H
        wl_in.append(P(f"wl{l}", [din, H], f32, isOutput=False))
        wr_in.append(P(f"wr{l}", [din, H], f32, isOutput=False))
        att_in.append(P(f"att{l}", [1, H], f32, isOutput=False))
        bl_in.append(P(f"bl{l}", [1, H], f32, isOutput=False))
        br_in.append(P(f"br{l}", [1, H], f32, isOutput=False))
        bb_in.append(P(f"bb{l}", [1, H], f32, isOutput=False))
    lw1_in = P("lw1", [H, H], f32, isOutput=False)
    lb1_in = P("lb1", [1, H], f32, isOutput=False)
    lw2_in = P("lw2", [H, 256], f32, isOutput=False)
    lb2_in = P("lb2", [1, 256], f32, isOutput=False)
    lw3_in = P("lw3", [256, 2], f32, isOutput=False)
    lb3_in = P("lb3", [2, 1], f32, isOutput=False)
    srcidx_in = P("srcidx", [128, NT * 8], i16, isOutput=False)
    dstidx_in = P("dstidx", [128, NT * 8], i16, isOutput=False)
    dstval_in = P("dstval", [128, NT], f32, isOutput=False)
    iota_in = P("iota", [128, BLK], f32, isOutput=False)
    sgn_in = P("sgn", [2, 1], f32, isOutput=False)
    logitsT_out = P("logitsT", [2, NPAD], f32, isOutput=True)
    probs0_out = P("probs0", [1, NPAD], f32, isOutput=True)
    probs1_out = P("probs1", [1, NPAD], f32, isOutput=True)

    x_bf = nc.dram_tensor("x_bf", [NPAD, DIN], bf16)
    xl_loc, xl_full, xr_dr = [], [], []
    for l in range(4):
        xl_loc.append(nc.dram_tensor(f"xlloc{l}", [NPAD, XLW], bf16))
        xl_full.append(nc.dram_tensor(f"xlfull{l}", [NC * NPAD, XLW], bf16,
                                      addr_space="Shared"))
        xr_dr.append(nc.dram_tensor(f"xr{l}", [NPAD, H], bf16))
    h_dr = [nc.dram_tensor(f"h{l}", [NPAD, H], bf16) for l in range(4)]

    with tile.TileContext(nc) as tc, ExitStack() as ctx:
        wp = ctx.enter_context(tc.tile_pool(name="wp", bufs=1))
        np_ = ctx.enter_context(tc.tile_pool(name="np", bufs=2))
        ep = ctx.enter_context(tc.tile_pool(name="ep", bufs=3))
        gp = ctx.enter_context(tc.tile_pool(name="gp", bufs=2))
        ps = ctx.enter_context(tc.tile_pool(name="ps", bufs=2, space="PSUM"))

        # ---------------- constants, indices ----------------
        ones128 = wp.tile([128, 1], bf16, tag="ones128")
        nc.vector.memset(ones128[:, :], 1.0)
        onesrow = wp.tile([1, 128], bf16, tag="onesrow")
        nc.vector.memset(onesrow[:1, :], 1.0)
        sgn = wp.tile([2, 1], f32, tag="sgn")
        nc.sync.dma_start(out=sgn[:2, :], in_=sgn_in[:, :])
        srcidx = wp.tile([128, NT * 8], i16, tag="srcidx")
        nc.sync.dma_start(out=srcidx[:, :], in_=srcidx_in[:, :])
        dstidx = wp.tile([128, NT * 8], i16, tag="dstidx")
        nc.sync.dma_start(out=dstidx[:, :], in_=dstidx_in[:, :])
        dstval = wp.tile([128, NT], f32, tag="dstval")
        nc.sync.dma_start(out=dstval[:, :], in_=dstval_in[:, :])
        iotaf = wp.tile([128, BLK], f32, tag="iotaf")
        nc.sync.dma_start(out=iotaf[:, :], in_=iota_in[:, :])
        iota = wp.tile([128, BLK], bf16, tag="iota")
        nc.vector.tensor_copy(iota[:, :], iotaf[:, :])

        # ---------------- weight prep (SBUF-resident) ----------------
        attb, blb_row, brb_row, recipcol = [], [], [], []
        for l in range(4):
            ab = wp.tile([128, H], f32, tag=f"attb{l}")
            nc.sync.dma_start(out=ab[:, :], in_=att_in[l][:, :].broadcast_to((128, H)))
            nc.scalar.activation(ab[:, :], ab[:, :], AF.Abs)
            nc.vector.tensor_scalar_max(ab[:, :], ab[:, :], 1e-30)
            attb.append(ab)
            rc = wp.tile([128, H // 128], f32, tag=f"rc{l}")
            nc.sync.dma_start(out=rc[:, :],
                              in_=att_in[l][0, :].rearrange("(k p) -> p k", p=128))
            nc.scalar.activation(rc[:, :], rc[:, :], AF.Abs)
            nc.vector.tensor_scalar_max(rc[:, :], rc[:, :], 1e-30)
            rcr = wp.tile([128, H // 128], f32, tag=f"rcr{l}")
            nc.vector.reciprocal(rcr[:, :], rc[:, :])
            recipcol.append(rcr)
            if use_bias:
                trow = np_.tile([1, H], f32, tag="brow_ld", bufs=1)
                nc.sync.dma_start(out=trow[:1, :], in_=bl_in[l][:, :])
                trow2 = np_.tile([1, H], f32, tag="brow_ld2", bufs=1)
                nc.sync.dma_start(out=trow2[:1, :], in_=bb_in[l][:, :])
                tsum = np_.tile([1, H], f32, tag="brow_sum", bufs=1)
                nc.vector.tensor_add(tsum[:1, :], trow[:1, :], trow2[:1, :])
                blr = wp.tile([1, H], bf16, tag=f"blb{l}")
                nc.vector.tensor_mul(blr[:1, :], tsum[:1, :], ab[0:1, :])
                blb_row.append(blr)
                trow3 = np_.tile([1, H], f32, tag="brow_ld3", bufs=1)
                nc.sync.dma_start(out=trow3[:1, :], in_=br_in[l][:, :])
                tdif = np_.tile([1, H], f32, tag="brow_dif", bufs=1)
                nc.vector.tensor_sub(tdif[:1, :], trow3[:1, :], trow2[:1, :])
                brr = wp.tile([1, H], bf16, tag=f"brb{l}")
                nc.vector.tensor_mul(brr[:1, :], tdif[:1, :], ab[0:1, :])
                brb_row.append(brr)
            else:
                blb_row.append(None)
                brb_row.append(None)

        # GAT weights -> SBUF, col-scaled by |att_l|, rows by 1/|att_{l-1}|
        wld_sb, wrd_sb = [], []
        for l in range(4):
            din = DIN if l == 0 else H
            nk0 = din // 128
            for W_in, lst, nm in ((wl_in[l], wld_sb, "wl"), (wr_in[l], wrd_sb, "wr")):
                wsb = wp.tile([128, nk0, H], bf16, tag=f"{nm}d{l}")
                for k0 in range(0, nk0, 2):
                    kw = min(2, nk0 - k0)
                    wt = np_.tile([128, 2, H], f32, tag="wprep", bufs=1)
                    nc.sync.dma_start(
                        out=wt[:, :kw, :],
                        in_=W_in[k0 * 128:(k0 + kw) * 128, :].rearrange(
                            "(k p) h -> p k h", p=128))
                    for kk in range(kw):
                        k = k0 + kk
                        if l == 0:
                            nc.vector.tensor_mul(wsb[:, k, :], wt[:, kk, :], attb[l][:, :])
                        else:
                            wt2 = np_.tile([128, H], f32, tag="wprep2", bufs=2)
                            nc.vector.tensor_mul(wt2[:, :], wt[:, kk, :], attb[l][:, :])
                            nc.vector.tensor_scalar_mul(wsb[:, k, :], wt2[:, :],
                                                        recipcol[l - 1][:, k:k + 1])
                lst.append(wsb)

        # MLP weights
        lw1_dev = []
        for k in range(4):
            wt = np_.tile([128, H], f32, tag="wprep1", bufs=1)
            nc.sync.dma_start(out=wt[:, :], in_=lw1_in[ts(k, 128), :])
            wdev = wp.tile([128, H], bf16, tag=f"lw1_{k}")
            nc.vector.tensor_scalar_mul(wdev[:, :], wt[:, :], recipcol[3][:, k:k + 1])
            lw1_dev.append(wdev)
        lw2_dev = []
        for k in range(4):
            wt = np_.tile([128, 256], f32, tag="wprep1", bufs=1)
            nc.sync.dma_start(out=wt[:, :], in_=lw2_in[ts(k, 128), :])
            wdev = wp.tile([128, 256], bf16, tag=f"lw2_{k}")
            nc.vector.tensor_copy(wdev[:, :], wt[:, :])
            lw2_dev.append(wdev)
        lw3_dev = []
        for k in range(2):
            wt3 = np_.tile([128, 2], f32, tag="wprep3", bufs=1)
            nc.sync.dma_start(out=wt3[:, :], in_=lw3_in[ts(k, 128), :])
            wdev = wp.tile([128, 2], bf16, tag=f"lw3_{k}")
            nc.vector.tensor_copy(wdev[:, :], wt3[:, :])
            lw3_dev.append(wdev)
        lb1col = wp.tile([128, 4], f32, tag="lb1c")
        nc.sync.dma_start(out=lb1col[:, :], in_=lb1_in[0, :].rearrange("(k p) -> p k", p=128))
        lb2col = wp.tile([128, 2], f32, tag="lb2c")
        nc.sync.dma_start(out=lb2col[:, :], in_=lb2_in[0, :].rearrange("(k p) -> p k", p=128))
        lb3col = wp.tile([2, 1], f32, tag="lb3c")
        nc.sync.dma_start(out=lb3col[:2, :], in_=lb3_in[:, :])

        # x: fp32 -> bf16 cast for transposes
        nc.gpsimd.dma_start(out=x_bf[:, :], in_=x_in[:, :])

        # hT tiles (feature-major activations), persistent tags
        nkmax = DIN // 128
        hT = [np_.tile([128, NPAD], bf16, tag=f"hT{k}", name=f"hT{k}", bufs=1)
              for k in range(nkmax)]
        for k in range(nkmax):
            nc.sync.dma_start(out=hT[k][:, :], in_=x_bf[:, ts(k, 128)], transpose=True)

        # ---------------- layers (node l+1 interleaved into edge l) ----
        def node_mm(lw, m):
            """PE part of the table build for chunk m of layer lw."""
            nkw = (DIN if lw == 0 else H) // 128
            sl = slice(m * 128, (m + 1) * 128)
            pxl = ps.tile([128, H], f32, tag="pnl")
            pxr = ps.tile([128, H], f32, tag="pnr")
            last = not use_bias
            for k in range(nkw):
                st = (k == 0)
                sp = last and (k == nkw - 1)
                nc.tensor.matmul(pxl[:, :], hT[k][:, sl], wld_sb[lw][:, k, :],
                                 start=st, stop=sp, skip_group_check=True)
                nc.tensor.matmul(pxr[:, :], hT[k][:, sl], wrd_sb[lw][:, k, :],
                                 start=st, stop=sp, skip_group_check=True)
            if use_bias:
                nc.tensor.matmul(pxl[:, :], onesrow[:1, :], blb_row[lw][:1, :],
                                 start=False, stop=True, skip_group_check=True)
                nc.tensor.matmul(pxr[:, :], onesrow[:1, :], brb_row[lw][:1, :],
                                 start=False, stop=True, skip_group_check=True)
            return pxl, pxr

        def node_stage(lw, m, pxl, pxr):
            """Staging (copies + a-col + DMA out) for chunk m of layer lw."""
            kpw = KP[lw]
            xl_sb = np_.tile([128, XLW], bf16, tag="xlsb", bufs=2)
            xr_sb = np_.tile([128, H], bf16, tag="xrsb", bufs=2)
            A1 = np_.tile([128, 1], f32, tag="A1", bufs=2)
            A2 = np_.tile([128, 1], f32, tag="A2", bufs=2)
            if kpw > 0:
                nc.scalar.activation(xl_sb[:, :kpw], pxl[:, :kpw], AF.Copy,
                                     accum_out=A1[:, :])
            else:
                nc.vector.memset(A1[:, :], 0.0)
            if kpw < H:
                nc.scalar.activation(xl_sb[:, kpw:H], pxl[:, kpw:], AF.Copy,
                                     accum_out=A2[:, :])
            else:
                nc.vector.memset(A2[:, :], 0.0)
            dA = np_.tile([128, 1], f32, tag="dA", bufs=2)
            nc.vector.tensor_sub(dA[:, :], A1[:, :], A2[:, :])
            nc.vector.tensor_scalar_mul(xl_sb[:, H:H + 1], dA[:, :], 0.6)
            nc.vector.tensor_copy(xr_sb[:, :], pxr[:, :])
            nc.sync.dma_start(out=xl_loc[lw][m * 128:(m + 1) * 128, :],
                              in_=xl_sb[:, :])
            nc.sync.dma_start(out=xr_dr[lw][m * 128:(m + 1) * 128, :],
                              in_=xr_sb[:, :])

        def node_chunk(lw, m):
            pxl, pxr = node_mm(lw, m)
            node_stage(lw, m, pxl, pxr)

        def all_gather(lw):
            if single_core:
                nc.sync.dma_start(out=xl_full[lw][0:NPAD, :], in_=xl_loc[lw][:, :])
            else:
                nc.gpsimd.collective_compute(
                    "AllGather", ALU.bypass,
                    replica_groups=[list(range(NC))],
                    ins=[xl_loc[lw][:, :]], outs=[xl_full[lw][:, :]],
                )

        for m in range(NBLK):
            node_chunk(0, m)
        all_gather(0)

        for l in range(4):
            kp = KP[l]
            for b in range(NBLK):
                if l < 3 and b >= 5:
                    pxlr = node_mm(l + 1, b - 5)
                TB = TBs[b]
                c0 = cum[b]
                nidx = TB * 128
                xlg = gp.tile([128, TBMAX, XLW], bf16, tag="xlg")
                nc.gpsimd.dma_gather(
                    out_ap=xlg[:, :TB, :], in_ap=xl_full[l][:, :],
                    idxs_ap=srcidx[:, c0 * 8:(c0 + TB) * 8],
                    num_idxs=nidx, num_idxs_reg=nidx, elem_size=XLW,
                    single_packet=False)
                xrg = gp.tile([128, TBMAX, H], bf16, tag="xrg")
                nc.gpsimd.dma_gather(
                    out_ap=xrg[:, :TB, :], in_ap=xr_dr[l][:, :],
                    idxs_ap=dstidx[:, c0 * 8:(c0 + TB) * 8],
                    num_idxs=nidx, num_idxs_reg=nidx, elem_size=H,
                    single_packet=False, queue_num=1)
                ubuf = gp.tile([128, TBMAX, H], bf16, tag="ubuf")
                Pcol = ep.tile([128, TBMAX], f32, tag="Pcol", bufs=2)
                nNc = ep.tile([128, TBMAX], f32, tag="nNc", bufs=2)
                tcol = ep.tile([128, TBMAX], f32, tag="tcol", bufs=2)
                tc2 = ep.tile([128, TBMAX], f32, tag="tc2", bufs=2)
                pbuf = ep.tile([128, TBMAX], f32, tag="pbuf", bufs=2)
                Sbuf = ep.tile([128, TBMAX, BLK], bf16, tag="Sbuf", bufs=1)
                pf = ps.tile([128, H], f32, tag="pf")
                ps1 = ps.tile([128, 1], f32, tag="pcol1")
                for t in range(TB):
                    nc.vector.tensor_add(ubuf[:, t, :], xlg[:, t, :H], xrg[:, t, :])
                for t in range(TB):
                    scratch = ep.tile([128, H], bf16, tag="scr", bufs=1)
                    if kp > 0:
                        nc.scalar.activation(scratch[:, :kp], ubuf[:, t, :kp], AF.Abs,
                                             accum_out=Pcol[:, t:t + 1])
                for t in range(TB):
                    if kp < H:
                        nc.vector.tensor_reduce(nNc[:, t:t + 1], ubuf[:, t, kp:],
                                                AX.X, ALU.add,
                                                apply_absolute_value=True, negate=True)
                if kp == 0:
                    nc.vector.memset(Pcol[:, :TB], 0.0)
                if kp == H:
                    nc.vector.memset(nNc[:, :TB], 0.0)
                nc.vector.tensor_add(tcol[:, :TB], Pcol[:, :TB], nNc[:, :TB])
                nc.vector.scalar_tensor_tensor(tc2[:, :TB], xlg[:, :TB, H:H + 1],
                                               2.5, tcol[:, :TB],
                                               ALU.mult, ALU.add)
                nc.scalar.activation(pbuf[:, :TB], tc2[:, :TB], AF.Exp, scale=0.4)
                for t in range(TB):
                    nc.vector.tensor_scalar(Sbuf[:, t, :], iota[:, :],
                                            dstval[:, c0 + t:c0 + t + 1],
                                            pbuf[:, t:t + 1],
                                            ALU.is_equal, ALU.mult)
                for t in range(TB):
                    nc.tensor.matmul(pf[:BLK, :], Sbuf[:, t, :], xlg[:, t, :H],
                                     start=(t == 0), stop=(t == TB - 1),
                                     skip_group_check=True)
                    nc.tensor.matmul(ps1[:BLK, :1], Sbuf[:, t, :], ones128[:, :1],
                                     start=(t == 0), stop=(t == TB - 1),
                                     skip_group_check=True)
                srec = ep.tile([128, 1], f32, tag="srec")
                nc.vector.reciprocal(srec[:BLK, :], ps1[:BLK, :1])
                hb = ep.tile([128, H], bf16, tag="hb", bufs=2)
                nc.scalar.activation(hb[:BLK, :], pf[:BLK, :], AF.Relu,
                                     scale=srec[:BLK, :])
                nc.sync.dma_start(out=h_dr[l][b * SLOT:b * SLOT + BLK, :],
                                  in_=hb[:BLK, :])
                if b == 4:
                    # first-half feature-major tiles; chunks 0-4 build during
                    # blocks 5-9
                    for k in range(4):
                        nc.sync.dma_start(out=hT[k][:, :640],
                                          in_=h_dr[l][:640, ts(k, 128)],
                                          transpose=True)
                if l < 3 and b >= 5:
                    node_stage(l + 1, b - 5, pxlr[0], pxlr[1])
            for k in range(4):
                nc.sync.dma_start(out=hT[k][:, 640:],
                                  in_=h_dr[l][640:, ts(k, 128)], transpose=True)
            if l < 3:
                for m in range(5, NBLK):
                    node_chunk(l + 1, m)
                all_gather(l + 1)

        # ---------------- MLP head (feature-major) ----------------
        jchunks = [(0, 512), (512, 512), (1024, 256)]
        h1T = [np_.tile([128, NPAD], bf16, tag=f"h1T{m}", name=f"h1T{m}", bufs=1)
               for m in range(4)]
        for m in range(4):
            for j0, w in jchunks:
                pm = ps.tile([128, H], f32, tag="pf")
                for k in range(4):
                    nc.tensor.matmul(pm[:, :w], lw1_dev[k][:, ts(m, 128)],
                                     hT[k][:, j0:j0 + w], start=(k == 0), stop=(k == 3),
                                     skip_group_check=True)
                nc.scalar.activation(h1T[m][:, j0:j0 + w], pm[:, :w], AF.Relu,
                                     bias=lb1col[:, m:m + 1])
        h2T = [np_.tile([128, NPAD], bf16, tag=f"h2T{m}", name=f"h2T{m}", bufs=1)
               for m in range(2)]
        for m in range(2):
            for j0, w in jchunks:
                pm = ps.tile([128, H], f32, tag="pf")
                for k in range(4):
                    nc.tensor.matmul(pm[:, :w], lw2_dev[k][:, ts(m, 128)],
                                     h1T[k][:, j0:j0 + w], start=(k == 0), stop=(k == 3),
                                     skip_group_check=True)
                nc.scalar.activation(h2T[m][:, j0:j0 + w], pm[:, :w], AF.Relu,
                                     bias=lb2col[:, m:m + 1])
        logT = np_.tile([2, NPAD], f32, tag="logT", bufs=1)
        p0 = np_.tile([1, NPAD], f32, tag="p0", bufs=1)
        p1 = np_.tile([1, NPAD], f32, tag="p1", bufs=1)
        for j0, w in jchunks:
            pm = ps.tile([128, H], f32, tag="pf")
            for k in range(2):
                nc.tensor.matmul(pm[:2, :w], lw3_dev[k][:, :],
                                 h2T[k][:, j0:j0 + w], start=(k == 0), stop=(k == 1),
                                 skip_group_check=True)
            nc.scalar.activation(logT[:2, j0:j0 + w], pm[:2, :w], AF.Identity,
                                 bias=lb3col[:2, :])
            pd = ps.tile([128, H], f32, tag="pf")
            nc.tensor.matmul(pd[:1, :w], sgn[:2, :], logT[:2, j0:j0 + w],
                             start=True, stop=True, skip_group_check=True)
            nc.scalar.activation(p1[:1, j0:j0 + w], pd[:1, :w], AF.Sigmoid)
            nc.scalar.activation(p0[:1, j0:j0 + w], pd[:1, :w], AF.Sigmoid, scale=-1.0)
        nc.sync.dma_start(out=logitsT_out[:, :], in_=logT[:2, :])
        nc.sync.dma_start(out=probs0_out[:, :], in_=p0[:1, :])
        nc.sync.dma_start(out=probs1_out[:, :], in_=p1[:1, :])

    nc.compile()
    return nc


_CACHE = {}
_LAST_IN_MAPS = None


def _get_program(TBs, KP, use_bias):
    key = (tuple(TBs), tuple(KP), use_bias)
    if key not in _CACHE:
        _CACHE[key] = _build(TBs, KP, use_bias)
    return _CACHE[key]


def _run(inputs, trace=False):
    inp = {k: np.asarray(v) for k, v in inputs.items()}
    x = inp["x"].astype(np.float32)
    edge_index = inp["edge_index"].astype(np.int64)
    TBs, cores, assign = _prep_edges(edge_index)

    perms, KP = [], []
    for l in range(1, 5):
        att = inp[f"att{l}"].astype(np.float32)
        perm = np.argsort(att <= 0, kind="stable")
        perms.append(perm)
        KP.append(int((att > 0).sum()))
    use_bias = any(
        np.abs(inp[k]).max() > 0
        for k in ("bl1", "br1", "b1", "bl2", "br2", "b2",
                  "bl3", "br3", "b3", "bl4", "br4", "b4"))
    w_host = {}
    for l in range(4):
        rowp = perms[l - 1] if l > 0 else None
        for nm in ("wl", "wr"):
            W = inp[f"{nm}{l + 1}"].astype(np.float32)
            if rowp is not None:
                W = W[rowp, :]
            w_host[f"{nm}{l}"] = np.ascontiguousarray(W[:, perms[l]])
        for nm, key in (("att", "att"), ("bl", "bl"), ("br", "br"), ("bb", "b")):
            v = inp[f"{key}{l + 1}"].astype(np.float32)[perms[l]]
            w_host[f"{nm}{l}"] = v.reshape(1, H).copy()
    lw1 = np.ascontiguousarray(inp["lw1"].astype(np.float32)[perms[3], :])

    x_pad = np.zeros((NC, NPAD, DIN), np.float32)
    for c in range(NC):
        for b in range(NBLK):
            x_pad[c, b * SLOT:b * SLOT + BLK] = x[assign[c][b * BLK:(b + 1) * BLK]]

    iota = np.tile(np.arange(BLK, dtype=np.float32), (128, 1)).copy()
    ncprog = _get_program(TBs, KP, use_bias)
    in_maps = []
    for c in range(NC):
        m = {"x": x_pad[c],
             "lw1": lw1, "lb1": inp["lb1"].astype(np.float32).reshape(1, H),
             "lw2": inp["lw2"].astype(np.float32),
             "lb2": inp["lb2"].astype(np.float32).reshape(1, 256),
             "lw3": inp["lw3"].astype(np.float32),
             "lb3": inp["lb3"].astype(np.float32).reshape(2, 1),
             "sgn": np.array([[-1.0], [1.0]], np.float32),
             "srcidx": cores[c]["src16"], "dstidx": cores[c]["dst16"],
             "dstval": cores[c]["dstval"], "iota": iota}
        m.update(w_host)
        in_maps.append(m)

    global _LAST_IN_MAPS
    _LAST_IN_MAPS = in_maps
    res = run_bass_kernel_spmd(ncprog, in_maps, list(range(NC)), trace=trace)
    logits = np.empty((N, 2), np.float32)
    probs = np.empty((N, 2), np.float32)
    slot_rows = np.concatenate([np.arange(b * SLOT, b * SLOT + BLK)
                                for b in range(NBLK)])
    for c in range(NC):
        r = res.results[c]
        logits[assign[c]] = r["logitsT"].T[slot_rows]
        probs[assign[c], 0] = r["probs0"][0][slot_rows]
        probs[assign[c], 1] = r["probs1"][0][slot_rows]
    return (logits, probs), res


def kernel(**inputs):
    out, _ = _run(inputs, trace=False)
    return out
